# revision 39
# baseline (speedup 1.0000x reference)
"""Trainium2 Bass kernel for the combined focal loss (8-core data parallel).

Sharding: batch dim B=16 split 2 samples/core across 8 cores. Each core
computes partial sums of the heavy loss terms; the host combines the
(tiny) partials in float64. The pairwise-cosine term is reduced
algebraically:

    pos_sum - neg_sum = 0.5 * (s_pos . s_neg - ||s_pos||^2)

with s_pos/s_neg sums of row-normalized flattened heatmaps, so each core
only returns its local unit-row sum [128, 288] (fp16) and no collective
is needed.

The dominant traffic is cstency_preds, downcast host-side to fp8-e4m3
(quantization rel-err ~2e-3 on the loss, an order under the 2e-2 gate).
It streams on the sync-engine HWDGE ring in 8 chunks sized so the tail
chunk is tiny (4 blocks): after the last byte lands, only ~4 matmuls +
one small sigmoid + two small vector ops + the stats DMA remain on the
critical path. Heatmaps ship as fp8 too: x raw plus (1-g)^4
precomputed on host (so the focal neg weight needs no device ops), with
feat/rn as a separate 4-col fp16 tensor. The scalar engine orders its
work Sigmoid/Sigmoid/Ln so only 3 ACT table loads happen and the
Sigmoid table is resident again before the stream sigmoids.

Partial sums ship raw as stats[128, 16] (host sums partitions) -- no
PE reduction on the tail. s_vec ships as two fp16 half-DMAs so no
single DMA queue carries the whole output (a straggler queue delayed
the stream by ~1 us in the 1-DMA f32 version).

The de-minimis scalar terms (cls/temporal BCE over 16x1/16x8 inputs,
and the focal pos-term, which touches only the ~2 elements/sample where
gt==1.0) are computed on host in float64, like the argmax/row-norm prep.
"""

import numpy as np
import ml_dtypes

import concourse.bacc as bacc
import concourse.tile as tile
from concourse import mybir

F32 = mybir.dt.float32
F16 = mybir.dt.float16
F8 = mybir.dt.float8e4
AF = mybir.ActivationFunctionType
ALU = mybir.AluOpType

B = 16
H = W = 192
HW = H * W            # 36864
D = 64
NCORES = 8
SPC = B // NCORES     # 2 samples per core
P = 128
HMF = HW // P         # 288 cols per sample in [128, .] layout
NF = SPC * HMF        # 576
OFFW = SPC * 2 * HW // P   # 1152 cols (2 samples x 2 ch)
FL_EPS = 0.1
NOISE = 0.2
POSC = (1.0 - FL_EPS) + FL_EPS * NOISE   # 0.92

# stream chunk sizes in 128-col blocks. Uniform small chunks: the
# per-chunk matmul batch trails its chunk's DMA by ~1 chunk, so big
# chunks pile up ~4 us of serialized matmuls after the stream ends;
# 18-block chunks keep the backlog under ~1 us while the sync-seq can
# still generate descriptors (0.62 us/chunk) faster than the queues
# drain them (0.83 us/chunk).
# First chunk small so the PE's 14 us back-to-back matmul stream starts
# as early as possible; last chunk small so the post-stream serial tail
# (matmul+sigmoid+dif+dsq) is short.
CHUNKS = [12] + [18] * 15 + [6]
assert sum(CHUNKS) == HW // P
NG = len(CHUNKS)

# stats tile columns (per-partition partials; host sums partitions).
# The consistency term uses sum((enc-gt)^2) = sum(enc^2) - 2*sum(enc*gt)
# + sum(gt^2): enc^2 accumulates on the scalar engine (Square is in
# every ACT table, so no table reload), enc*gt on vector, gt^2 on host.
ST_NEG = 0      # sum log(1-p) * p^2 * (1-g)^4  == neg_s
ST_OFFSQ = 1    # sum ((p-g)*c)^2
ST_OFFN = 2     # sum c
ST_E2 = 3               # NG cols, per-chunk sum enc^2
ST_EG = ST_E2 + NG      # NG cols, per-chunk sum enc*gt
ST_W = 40
assert ST_EG + NG <= ST_W

FRW = 256       # feat/rn tensor width (padded, see fr16 below)


IN_GT = 0                      # cst gts, pretransposed [p, blk, s]
IN_OFF = IN_GT + HMF * SPC     # offset [preds | gts]
IN_XW = IN_OFF + 2 * OFFW      # heatmap [x | (1-g)^4]
IN_W = IN_XW + 2 * NF          # 4032 fp8 cols total


def build_nc():
    nc = bacc.Bacc(None, target_bir_lowering=False)

    # ALL fp8 inputs packed into one tensor [gt | off | xw] shipped as
    # the FIRST DMA on the sync ring, ahead of the stream chunks. The
    # 16 HW DMA queues round-robin between logical rings, so inputs on
    # any other ring get ~15% duty against the stream and land as late
    # as ~21 us (gating the tail). FIFO order on the stream's own ring
    # guarantees they land first, at full bandwidth (~1.3 us).
    inp8 = nc.dram_tensor("inp8", [P, IN_W], F8, kind="ExternalInput")
    # feat (2 cols, block-diag by sample) + rn (2 cols) + zero padding,
    # fp16 (padded to 512 B/partition to dodge the tiny-packet DMA
    # path); rides the otherwise-idle gpsimd SWDGE ring, needed only
    # by the matmuls/sv which have slack.
    fr16 = nc.dram_tensor("fr16", [P, FRW], F16, kind="ExternalInput")
    cst_p = nc.dram_tensor("cst_p", [P, HW], F8, kind="ExternalInput")

    s_vec = nc.dram_tensor("s_vec", [P, HMF], F16, kind="ExternalOutput")
    stats = nc.dram_tensor("stats", [P, ST_W], F32, kind="ExternalOutput")

    with tile.TileContext(nc, pool_alloc_mode="queue") as tc:
        with (
            tc.tile_pool(name="consts", bufs=1) as consts,
            tc.tile_pool(name="cstp", bufs=NG) as cstp,
            tc.tile_pool(name="encp", bufs=10) as encp,
            tc.tile_pool(name="hmp", bufs=1) as hmp,
            tc.tile_pool(name="offp", bufs=1) as offp,
            tc.tile_pool(name="ps_stream", bufs=8, space="PSUM") as ps_stream,
        ):
            # fr (feat for the matmuls) and chunk0's DMA go FIRST, even
            # before the packed inputs: the tensor engine is ~100% busy
            # for 288 x ~48ns = 13.8 us of back-to-back matmuls, so
            # every us the first chunk + feat land earlier moves the
            # whole matmul stream (and with it the tail) up. On the
            # gpsimd SWDGE ring fr pays ~1 us first-byte latency, so it
            # rides the sync ring ahead of chunk0 (0.25 us at full BW).
            fr = consts.tile([P, FRW], F16)
            nc.sync.dma_start(out=fr, in_=fr16[:, :])
            cw0 = CHUNKS[0] * P
            t_first = cstp.tile([P, cw0], F8, name="t0", tag="t")
            nc.sync.dma_start(out=t_first, in_=cst_p[:, 0:cw0])
            inp = hmp.tile([P, IN_W], F8)
            nc.sync.dma_start(out=inp, in_=inp8[:, :])

            st = consts.tile([P, ST_W], F32)

            gt_sb = inp[:, IN_GT:IN_GT + HMF * SPC]
            off_sb = inp[:, IN_OFF:IN_OFF + 2 * OFFW]
            xw = inp[:, IN_XW:IN_XW + 2 * NF]
            xf = xw[:, 0:NF]
            wf = xw[:, NF:2 * NF]
            feat_sb = fr[:, 0:2]
            rns_sb = fr[:, 2:4]
            w = {k: hmp.tile([P, NF], F16, tag=k, name=k)
                 for k in ("sp", "pt", "p2")}

            # scalar engine: both Sigmoids, then Ln, then the stream's
            # sigmoids -> 3 ACT table loads (no table holds both
            # Sigmoid and Ln), with the Sigmoid table resident again
            # before the stream so no mid-stream reload happens.
            # clip(p, 1e-4, 1-1e-4) is a no-op for |x| < 9.2, and
            # 1-p == sigmoid(-x) at table precision.
            nc.scalar.activation(w["sp"], xf, AF.Sigmoid, scale=-1.0)  # 1-p
            nc.scalar.activation(w["pt"], xf, AF.Sigmoid)              # p
            nc.scalar.activation(w["sp"], w["sp"], AF.Ln)       # log(1-p)

            # ---- offset partials, all on vector (everything finishes
            # by ~15 us, well before the stream tail needs the engine;
            # gpsimd is ~5x slower and would head-of-line-block oj) ----
            op_ = off_sb[:, :OFFW]
            og_ = off_sb[:, OFFW:]
            oc = offp.tile([P, OFFW], F16, tag="oc")
            nc.vector.tensor_scalar(
                out=oc, in0=og_, scalar1=0.0, scalar2=None, op0=ALU.is_gt,
            )                                                  # c
            nc.vector.reduce_sum(st[:, ST_OFFN:ST_OFFN + 1], oc[:],
                                 axis=mybir.AxisListType.X)
            od = offp.tile([P, OFFW], F16, tag="od")
            nc.vector.tensor_sub(od, op_, og_)                 # p - g
            om = offp.tile([P, OFFW], F16, tag="om")
            nc.vector.tensor_mul(om, od, oc)                   # (p-g)*c
            oj = offp.tile([P, OFFW], F16, tag="oj")
            nc.vector.scalar_tensor_tensor(
                out=oj, in0=om, scalar=1.0, in1=om,
                op0=ALU.mult, op1=ALU.mult,
                accum_out=st[:, ST_OFFSQ:ST_OFFSQ + 1],
            )

            # ---- heatmap focal neg partials, gated on Ln (~14 us); the
            # pos term and (1-g)^4 are host-side ----
            nc.vector.tensor_mul(w["p2"], w["pt"], w["pt"])    # p^2
            nc.vector.tensor_mul(w["p2"], w["sp"], w["p2"])    # sp*p^2
            nc.vector.scalar_tensor_tensor(
                out=w["p2"], in0=w["p2"], scalar=1.0, in1=wf,
                op0=ALU.mult, op1=ALU.mult,
                accum_out=st[:, ST_NEG:ST_NEG + 1],
            )

            # cosine partial: s_c = sum_s x_s * rn_s (ptr-scalar ops are
            # vector-only and need f32 scalars)
            rns32 = hmp.tile([P, SPC], F32, tag="rns32")
            nc.vector.tensor_copy(rns32, rns_sb)
            sv = hmp.tile([P, HMF], F16)
            nc.vector.tensor_scalar_mul(sv, xw[:, HMF:2 * HMF],
                                        rns32[:, 1:2])
            nc.vector.scalar_tensor_tensor(
                out=sv, in0=xw[:, 0:HMF], scalar=rns32[:, 0:1], in1=sv,
                op0=ALU.mult, op1=ALU.add,
            )

            # ---- consistency stream: sync ring, NG chunks ----
            col = 0
            for g, nb in enumerate(CHUNKS):
                cw = nb * P
                if g == 0:
                    t = t_first
                else:
                    t = cstp.tile([P, cw], F8, name="t%d" % g, tag="t")
                    nc.sync.dma_start(out=t, in_=cst_p[:, col:col + cw])
                pa = ps_stream.tile([P, nb, SPC], F32, tag="pa", name="pa")
                for j in range(nb):
                    nc.tensor.matmul(
                        pa[:, j, :], t[:, j * P:(j + 1) * P], feat_sb[:],
                        start=True, stop=True,
                    )
                enc = encp.tile([P, nb * SPC], F16, tag="enc", name="enc")
                nc.scalar.activation(
                    enc, pa.rearrange("p a b -> p (a b)"), AF.Sigmoid,
                    scale=0.125,
                )
                e2 = encp.tile([P, nb * SPC], F16, tag="e2", name="e2")
                nc.scalar.activation(
                    e2, enc, AF.Square,
                    accum_out=st[:, ST_E2 + g:ST_E2 + g + 1],
                )
                blk0 = col // P
                eg = encp.tile([P, nb * SPC], F16, tag="eg", name="eg")
                nc.vector.scalar_tensor_tensor(
                    out=eg, in0=enc, scalar=1.0,
                    in1=gt_sb[:, blk0 * SPC:(blk0 + nb) * SPC],
                    op0=ALU.mult, op1=ALU.mult,
                    accum_out=st[:, ST_EG + g:ST_EG + g + 1],
                )
                col += cw

            # s_vec ships AFTER the stream-sigmoid emissions: a
            # dma_start stalls its whole sequencer until the data is
            # ready, so putting these last on the scalar ring means
            # they can't delay the ACT table loads / stream sigmoids.
            # Two half-DMAs so no single queue carries the whole output.
            hh = HMF // 2
            nc.scalar.dma_start(out=s_vec[:, 0:hh], in_=sv[:, 0:hh])
            nc.scalar.dma_start(out=s_vec[:, hh:HMF], in_=sv[:, hh:HMF])

            # ship the raw per-partition stats; host sums the 128 rows
            nc.sync.dma_start(out=stats[:, :], in_=st)

    nc.finalize()
    return nc


def shard_inputs(hm_outputs, hm_targets, cls_preds, cls_gts,
                 offset_preds, offset_gts, cstency_preds, cstency_gts,
                 temp_loc_preds, temp_loc_gts):
    """Build the 8 per-core input maps + host-side fp64 scalar terms."""
    hm = np.ascontiguousarray(hm_outputs, np.float32).reshape(B, HW)
    hg = np.ascontiguousarray(hm_targets, np.float32).reshape(B, HW)
    hm8 = hm.astype(ml_dtypes.float8_e4m3)
    w4_8 = ((1.0 - hg) ** 4).astype(ml_dtypes.float8_e4m3)
    off = np.concatenate([
        np.ascontiguousarray(offset_preds, np.float32).reshape(B, 2 * HW),
        np.ascontiguousarray(offset_gts, np.float32).reshape(B, 2 * HW),
    ], axis=1).astype(ml_dtypes.float8_e4m3)     # [B, 4*HW] = [p | g]
    cp = np.ascontiguousarray(cstency_preds, np.float32).reshape(B, D, HW)
    cg = np.ascontiguousarray(cstency_gts, np.float32).reshape(B, HW)

    # rn from the QUANTIZED x so device unit-rows are self-consistent
    hm8f = hm8.astype(np.float64)
    rn = (1.0 / np.maximum(np.sqrt((hm8f ** 2).sum(axis=1)),
                           1e-6)).astype(np.float32)
    idx = np.argmax(cg, axis=-1)                       # [B]
    feat = cp[np.arange(B), :, idx]                    # [B, D] peak features
    cp8 = cp.astype(ml_dtypes.float8_e4m3)

    # ---- host fp64 de-minimis terms ----
    def bce_mean(x, y):
        x = x.astype(np.float64).ravel()
        y = y.astype(np.float64).ravel()
        sp = np.log1p(np.exp(-np.abs(x))) + np.maximum(x, 0.0)
        return float((sp - x * y).mean())

    loss_cls = bce_mean(np.asarray(cls_preds), np.asarray(cls_gts))
    loss_tmp = bce_mean(np.asarray(temp_loc_preds), np.asarray(temp_loc_gts))

    pos_mask = hg == 1.0
    num_pos = float(pos_mask.sum())
    xp = hm[pos_mask].astype(np.float64)
    pp = np.clip(1.0 / (1.0 + np.exp(-xp)), 1e-4, 1.0 - 1e-4)
    pos_s = float((POSC * np.log(pp) * (1.0 - pp) ** 2).sum())

    # sum(gt^2) over the SAME fp8-quantized gts the device sees, so the
    # expansion sum(enc^2) - 2*sum(enc*gt) + sum(gt^2) is consistent
    gt2 = float((cg.astype(ml_dtypes.float8_e4m3)
                 .astype(np.float64) ** 2).sum())

    host = {"loss_cls": loss_cls, "loss_tmp": loss_tmp,
            "num_pos": num_pos, "pos_s": pos_s, "gt2": gt2}

    in_maps = []
    for c in range(NCORES):
        b0 = c * SPC
        # packed fp8 inputs [gt | off | xw]; gt pre-transposed to the
        # matmul output layout: gt[p, blk*SPC + s] = cg[b0+s, blk*128+p]
        pk = np.empty((P, IN_W), ml_dtypes.float8_e4m3)
        pk[:, IN_GT:IN_GT + HMF * SPC] = np.ascontiguousarray(
            cg[b0:b0 + SPC].reshape(SPC, HMF, P).transpose(2, 1, 0)
        ).reshape(P, HMF * SPC).astype(ml_dtypes.float8_e4m3)
        pk[:, IN_OFF:IN_OFF + 2 * OFFW] = off[b0:b0 + SPC].reshape(
            P, 2 * OFFW)
        pk[:, IN_XW:IN_XW + NF] = hm8[b0:b0 + SPC].reshape(
            SPC, P, HMF).transpose(1, 0, 2).reshape(P, NF)
        pk[:, IN_XW + NF:IN_XW + 2 * NF] = w4_8[b0:b0 + SPC].reshape(
            SPC, P, HMF).transpose(1, 0, 2).reshape(P, NF)
        fr = np.zeros((P, FRW), np.float16)
        for s in range(SPC):
            fr[s * D:(s + 1) * D, s] = feat[b0 + s].astype(np.float16)
        fr[:, 2:4] = np.tile(rn[b0:b0 + SPC].astype(np.float16), (P, 1))
        in_maps.append({
            "inp8": pk,
            "fr16": fr,
            "cst_p": cp8[b0:b0 + SPC].reshape(P, HW),
        })
    return in_maps, host


def combine_outputs(results, host):
    """results: list of 8 per-core {'s_vec': [128,288] f16,
    'stats': [128,16] f32 per-partition partials}."""
    stc = np.stack([r["stats"].astype(np.float64).sum(axis=0)
                    for r in results])
    col = stc.sum(axis=0)                            # [16] over cores
    neg_s = col[ST_NEG]                  # device stores sum log(1-p)*p^2*w
    off_sq = col[ST_OFFSQ]
    off_n = col[ST_OFFN]
    cst_sq = (col[ST_E2:ST_E2 + NG].sum()
              - 2.0 * col[ST_EG:ST_EG + NG].sum() + host["gt2"])

    num_pos, pos_s = host["num_pos"], host["pos_s"]
    if num_pos == 0:
        loss_hm = -neg_s
    else:
        loss_hm = -(pos_s + neg_s) / max(num_pos, 1.0)
    svs = [r["s_vec"].reshape(-1).astype(np.float64) for r in results]
    h = B // 2
    s_pos = sum(svs[:NCORES // 2])
    s_neg = sum(svs[NCORES // 2:])
    loss_dst = 0.5 * (s_pos @ s_neg - s_pos @ s_pos) / (h * h) * 0.1
    loss_off = 0.5 * (off_sq / (B * 2 * HW)) / (off_n + 1e-6)
    loss_cst = cst_sq / (B * HW) * 0.1
    return np.array([loss_hm, host["loss_cls"], loss_dst, loss_off,
                     loss_cst, host["loss_tmp"]], np.float32)


_CACHE = {}


def kernel(**inputs):
    from concourse.bass_utils import run_bass_kernel_spmd
    if "nc" not in _CACHE:
        _CACHE["nc"] = build_nc()
    nc = _CACHE["nc"]
    in_maps, host = shard_inputs(**inputs)
    res = run_bass_kernel_spmd(nc, in_maps, core_ids=list(range(NCORES)))
    return combine_outputs(res.results, host)


# revision 41
# speedup vs baseline: 1.0106x; 1.0106x over previous
"""Trainium2 Bass kernel for the combined focal loss (8-core data parallel).

Sharding: batch dim B=16 split 2 samples/core across 8 cores. Each core
computes partial sums of the heavy loss terms; the host combines the
(tiny) partials in float64. The pairwise-cosine term is reduced
algebraically:

    pos_sum - neg_sum = 0.5 * (s_pos . s_neg - ||s_pos||^2)

with s_pos/s_neg sums of row-normalized flattened heatmaps, so each core
only returns its local unit-row sum [128, 288] (fp16) and no collective
is needed.

The dominant traffic is cstency_preds, downcast host-side to fp8-e4m3
(quantization rel-err ~2e-3 on the loss, an order under the 2e-2 gate).
It streams on the sync-engine HWDGE ring in 8 chunks sized so the tail
chunk is tiny (4 blocks): after the last byte lands, only ~4 matmuls +
one small sigmoid + two small vector ops + the stats DMA remain on the
critical path. Heatmaps ship as fp8 too: x raw plus (1-g)^4
precomputed on host (so the focal neg weight needs no device ops), with
feat/rn as a separate 4-col fp16 tensor. The scalar engine orders its
work Sigmoid/Sigmoid/Ln so only 3 ACT table loads happen and the
Sigmoid table is resident again before the stream sigmoids.

Partial sums ship raw as stats[128, 16] (host sums partitions) -- no
PE reduction on the tail. s_vec ships as two fp16 half-DMAs so no
single DMA queue carries the whole output (a straggler queue delayed
the stream by ~1 us in the 1-DMA f32 version).

The de-minimis scalar terms (cls/temporal BCE over 16x1/16x8 inputs,
and the focal pos-term, which touches only the ~2 elements/sample where
gt==1.0) are computed on host in float64, like the argmax/row-norm prep.
"""

import numpy as np
import ml_dtypes

import concourse.bacc as bacc
import concourse.tile as tile
from concourse import mybir

F32 = mybir.dt.float32
F16 = mybir.dt.float16
F8 = mybir.dt.float8e4
AF = mybir.ActivationFunctionType
ALU = mybir.AluOpType

B = 16
H = W = 192
HW = H * W            # 36864
D = 64
NCORES = 8
SPC = B // NCORES     # 2 samples per core
P = 128
HMF = HW // P         # 288 cols per sample in [128, .] layout
NF = SPC * HMF        # 576
OFFW = SPC * 2 * HW // P   # 1152 cols (2 samples x 2 ch)
FL_EPS = 0.1
NOISE = 0.2
POSC = (1.0 - FL_EPS) + FL_EPS * NOISE   # 0.92

# stream chunk sizes in 128-col blocks. Uniform small chunks: the
# per-chunk matmul batch trails its chunk's DMA by ~1 chunk, so big
# chunks pile up ~4 us of serialized matmuls after the stream ends;
# 18-block chunks keep the backlog under ~1 us while the sync-seq can
# still generate descriptors (0.62 us/chunk) faster than the queues
# drain them (0.83 us/chunk).
# First chunk small so the PE's 14 us back-to-back matmul stream starts
# as early as possible; last chunk small so the post-stream serial tail
# (matmul+sigmoid+dif+dsq) is short.
CHUNKS = [12] + [18] * 15 + [6]
assert sum(CHUNKS) == HW // P
NG = len(CHUNKS)

# stats tile columns (per-partition partials; host sums partitions).
# The consistency term uses sum((enc-gt)^2) = sum(enc^2) - 2*sum(enc*gt)
# + sum(gt^2): enc^2 accumulates on the scalar engine (Square is in
# every ACT table, so no table reload), enc*gt on vector, gt^2 on host.
ST_NEG = 0      # sum log(1-p) * p^2 * (1-g)^4  == neg_s
ST_OFFSQ = 1    # sum ((p-g)*c)^2
ST_OFFN = 2     # sum c
ST_E2 = 3               # NG cols, per-chunk sum enc^2
ST_EG = ST_E2 + NG      # NG cols, per-chunk sum enc*gt
ST_W = 40
assert ST_EG + NG <= ST_W

FRW = 256       # feat/rn tensor width (padded, see fr16 below)


IN_GT = 0                      # cst gts, pretransposed [p, blk, s]
IN_OFF = IN_GT + HMF * SPC     # offset [preds | gts]
IN_XW = IN_OFF + 2 * OFFW      # heatmap [x | (1-g)^4]
IN_W = IN_XW + 2 * NF          # 4032 fp8 cols total


def build_nc():
    nc = bacc.Bacc(None, target_bir_lowering=False)

    # ALL fp8 inputs packed into one tensor [gt | off | xw] shipped as
    # the FIRST DMA on the sync ring, ahead of the stream chunks. The
    # 16 HW DMA queues round-robin between logical rings, so inputs on
    # any other ring get ~15% duty against the stream and land as late
    # as ~21 us (gating the tail). FIFO order on the stream's own ring
    # guarantees they land first, at full bandwidth (~1.3 us).
    inp8 = nc.dram_tensor("inp8", [P, IN_W], F8, kind="ExternalInput")
    # feat (2 cols, block-diag by sample) + rn (2 cols) + zero padding,
    # fp16 (padded to 512 B/partition to dodge the tiny-packet DMA
    # path); rides the otherwise-idle gpsimd SWDGE ring, needed only
    # by the matmuls/sv which have slack.
    fr16 = nc.dram_tensor("fr16", [P, FRW], F16, kind="ExternalInput")
    cst_p = nc.dram_tensor("cst_p", [P, HW], F8, kind="ExternalInput")

    s_vec = nc.dram_tensor("s_vec", [P, HMF], F16, kind="ExternalOutput")
    stats = nc.dram_tensor("stats", [P, ST_W], F32, kind="ExternalOutput")

    with tile.TileContext(nc, pool_alloc_mode="queue") as tc:
        with (
            tc.tile_pool(name="consts", bufs=1) as consts,
            tc.tile_pool(name="cstp", bufs=NG) as cstp,
            tc.tile_pool(name="encp", bufs=10) as encp,
            tc.tile_pool(name="hmp", bufs=1) as hmp,
            tc.tile_pool(name="offp", bufs=1) as offp,
            tc.tile_pool(name="ps_stream", bufs=8, space="PSUM") as ps_stream,
        ):
            # fr (feat for the matmuls) and chunk0's DMA go FIRST, even
            # before the packed inputs: the tensor engine is ~100% busy
            # for 288 x ~48ns = 13.8 us of back-to-back matmuls, so
            # every us the first chunk + feat land earlier moves the
            # whole matmul stream (and with it the tail) up. On the
            # gpsimd SWDGE ring fr pays ~1 us first-byte latency, so it
            # rides the sync ring ahead of chunk0 (0.25 us at full BW).
            fr = consts.tile([P, FRW], F16)
            nc.sync.dma_start(out=fr, in_=fr16[:, :])
            cw0 = CHUNKS[0] * P
            t_first = cstp.tile([P, cw0], F8, name="t0", tag="t")
            nc.sync.dma_start(out=t_first, in_=cst_p[:, 0:cw0])
            inp = hmp.tile([P, IN_W], F8)
            nc.sync.dma_start(out=inp, in_=inp8[:, :])

            st = consts.tile([P, ST_W], F32)

            gt_sb = inp[:, IN_GT:IN_GT + HMF * SPC]
            off_sb = inp[:, IN_OFF:IN_OFF + 2 * OFFW]
            xw = inp[:, IN_XW:IN_XW + 2 * NF]
            xf = xw[:, 0:NF]
            wf = xw[:, NF:2 * NF]
            feat_sb = fr[:, 0:2]
            rns_sb = fr[:, 2:4]
            w = {k: hmp.tile([P, NF], F16, tag=k, name=k)
                 for k in ("sp", "pt", "p2")}

            # scalar engine: both Sigmoids, then Ln, then the stream's
            # sigmoids -> 3 ACT table loads (no table holds both
            # Sigmoid and Ln), with the Sigmoid table resident again
            # before the stream so no mid-stream reload happens.
            # clip(p, 1e-4, 1-1e-4) is a no-op for |x| < 9.2, and
            # 1-p == sigmoid(-x) at table precision.
            nc.scalar.activation(w["sp"], xf, AF.Sigmoid, scale=-1.0)  # 1-p
            nc.scalar.activation(w["pt"], xf, AF.Sigmoid)              # p
            nc.scalar.activation(w["sp"], w["sp"], AF.Ln)       # log(1-p)

            # ---- offset partials, all on vector (everything finishes
            # by ~15 us, well before the stream tail needs the engine;
            # gpsimd is ~5x slower and would head-of-line-block oj) ----
            op_ = off_sb[:, :OFFW]
            og_ = off_sb[:, OFFW:]
            oc = offp.tile([P, OFFW], F16, tag="oc")
            nc.vector.tensor_scalar(
                out=oc, in0=og_, scalar1=0.0, scalar2=None, op0=ALU.is_gt,
            )                                                  # c
            nc.vector.reduce_sum(st[:, ST_OFFN:ST_OFFN + 1], oc[:],
                                 axis=mybir.AxisListType.X)
            od = offp.tile([P, OFFW], F16, tag="od")
            nc.vector.tensor_sub(od, op_, og_)                 # p - g
            om = offp.tile([P, OFFW], F16, tag="om")
            nc.vector.tensor_mul(om, od, oc)                   # (p-g)*c
            oj = offp.tile([P, OFFW], F16, tag="oj")
            nc.vector.scalar_tensor_tensor(
                out=oj, in0=om, scalar=1.0, in1=om,
                op0=ALU.mult, op1=ALU.mult,
                accum_out=st[:, ST_OFFSQ:ST_OFFSQ + 1],
            )

            # ---- heatmap focal neg partials, gated on Ln (~14 us); the
            # pos term and (1-g)^4 are host-side ----
            nc.vector.tensor_mul(w["p2"], w["pt"], w["pt"])    # p^2
            nc.vector.tensor_mul(w["p2"], w["sp"], w["p2"])    # sp*p^2
            nc.vector.scalar_tensor_tensor(
                out=w["p2"], in0=w["p2"], scalar=1.0, in1=wf,
                op0=ALU.mult, op1=ALU.mult,
                accum_out=st[:, ST_NEG:ST_NEG + 1],
            )

            # cosine partial: s_c = sum_s x_s * rn_s (ptr-scalar ops are
            # vector-only and need f32 scalars)
            rns32 = hmp.tile([P, SPC], F32, tag="rns32")
            nc.vector.tensor_copy(rns32, rns_sb)
            sv = hmp.tile([P, HMF], F16)
            nc.vector.tensor_scalar_mul(sv, xw[:, HMF:2 * HMF],
                                        rns32[:, 1:2])
            nc.vector.scalar_tensor_tensor(
                out=sv, in0=xw[:, 0:HMF], scalar=rns32[:, 0:1], in1=sv,
                op0=ALU.mult, op1=ALU.add,
            )

            # ---- consistency stream: sync ring, NG chunks ----
            col = 0
            for g, nb in enumerate(CHUNKS):
                cw = nb * P
                if g == 0:
                    t = t_first
                else:
                    t = cstp.tile([P, cw], F8, name="t%d" % g, tag="t")
                    nc.sync.dma_start(out=t, in_=cst_p[:, col:col + cw])
                pa = ps_stream.tile([P, nb, SPC], F32, tag="pa", name="pa")
                for j in range(nb):
                    nc.tensor.matmul(
                        pa[:, j, :], t[:, j * P:(j + 1) * P], feat_sb[:],
                        start=True, stop=True,
                    )
                enc = encp.tile([P, nb * SPC], F16, tag="enc", name="enc")
                nc.scalar.activation(
                    enc, pa.rearrange("p a b -> p (a b)"), AF.Sigmoid,
                    scale=0.125,
                )
                blk0 = col // P
                dif = encp.tile([P, nb * SPC], F16, tag="dif", name="dif")
                nc.vector.tensor_sub(
                    dif, enc, gt_sb[:, blk0 * SPC:(blk0 + nb) * SPC])
                dsq = encp.tile([P, nb * SPC], F16, tag="dsq", name="dsq")
                nc.vector.scalar_tensor_tensor(
                    out=dsq, in0=dif, scalar=1.0, in1=dif,
                    op0=ALU.mult, op1=ALU.mult,
                    accum_out=st[:, ST_E2 + g:ST_E2 + g + 1],
                )
                col += cw

            # s_vec ships AFTER the stream-sigmoid emissions: a
            # dma_start stalls its whole sequencer until the data is
            # ready, so putting these last on the scalar ring means
            # they can't delay the ACT table loads / stream sigmoids.
            # Two half-DMAs so no single queue carries the whole output.
            hh = HMF // 2
            nc.scalar.dma_start(out=s_vec[:, 0:hh], in_=sv[:, 0:hh])
            nc.scalar.dma_start(out=s_vec[:, hh:HMF], in_=sv[:, hh:HMF])

            # ship the raw per-partition stats; host sums the 128 rows
            nc.sync.dma_start(out=stats[:, :], in_=st)

    nc.finalize()
    return nc


def shard_inputs(hm_outputs, hm_targets, cls_preds, cls_gts,
                 offset_preds, offset_gts, cstency_preds, cstency_gts,
                 temp_loc_preds, temp_loc_gts):
    """Build the 8 per-core input maps + host-side fp64 scalar terms."""
    hm = np.ascontiguousarray(hm_outputs, np.float32).reshape(B, HW)
    hg = np.ascontiguousarray(hm_targets, np.float32).reshape(B, HW)
    hm8 = hm.astype(ml_dtypes.float8_e4m3)
    w4_8 = ((1.0 - hg) ** 4).astype(ml_dtypes.float8_e4m3)
    off = np.concatenate([
        np.ascontiguousarray(offset_preds, np.float32).reshape(B, 2 * HW),
        np.ascontiguousarray(offset_gts, np.float32).reshape(B, 2 * HW),
    ], axis=1).astype(ml_dtypes.float8_e4m3)     # [B, 4*HW] = [p | g]
    cp = np.ascontiguousarray(cstency_preds, np.float32).reshape(B, D, HW)
    cg = np.ascontiguousarray(cstency_gts, np.float32).reshape(B, HW)

    # rn from the QUANTIZED x so device unit-rows are self-consistent
    hm8f = hm8.astype(np.float64)
    rn = (1.0 / np.maximum(np.sqrt((hm8f ** 2).sum(axis=1)),
                           1e-6)).astype(np.float32)
    idx = np.argmax(cg, axis=-1)                       # [B]
    feat = cp[np.arange(B), :, idx]                    # [B, D] peak features
    cp8 = cp.astype(ml_dtypes.float8_e4m3)

    # ---- host fp64 de-minimis terms ----
    def bce_mean(x, y):
        x = x.astype(np.float64).ravel()
        y = y.astype(np.float64).ravel()
        sp = np.log1p(np.exp(-np.abs(x))) + np.maximum(x, 0.0)
        return float((sp - x * y).mean())

    loss_cls = bce_mean(np.asarray(cls_preds), np.asarray(cls_gts))
    loss_tmp = bce_mean(np.asarray(temp_loc_preds), np.asarray(temp_loc_gts))

    pos_mask = hg == 1.0
    num_pos = float(pos_mask.sum())
    xp = hm[pos_mask].astype(np.float64)
    pp = np.clip(1.0 / (1.0 + np.exp(-xp)), 1e-4, 1.0 - 1e-4)
    pos_s = float((POSC * np.log(pp) * (1.0 - pp) ** 2).sum())

    # sum(gt^2) over the SAME fp8-quantized gts the device sees, so the
    # expansion sum(enc^2) - 2*sum(enc*gt) + sum(gt^2) is consistent
    gt2 = float((cg.astype(ml_dtypes.float8_e4m3)
                 .astype(np.float64) ** 2).sum())

    host = {"loss_cls": loss_cls, "loss_tmp": loss_tmp,
            "num_pos": num_pos, "pos_s": pos_s, "gt2": gt2}

    in_maps = []
    for c in range(NCORES):
        b0 = c * SPC
        # packed fp8 inputs [gt | off | xw]; gt pre-transposed to the
        # matmul output layout: gt[p, blk*SPC + s] = cg[b0+s, blk*128+p]
        pk = np.empty((P, IN_W), ml_dtypes.float8_e4m3)
        pk[:, IN_GT:IN_GT + HMF * SPC] = np.ascontiguousarray(
            cg[b0:b0 + SPC].reshape(SPC, HMF, P).transpose(2, 1, 0)
        ).reshape(P, HMF * SPC).astype(ml_dtypes.float8_e4m3)
        pk[:, IN_OFF:IN_OFF + 2 * OFFW] = off[b0:b0 + SPC].reshape(
            P, 2 * OFFW)
        pk[:, IN_XW:IN_XW + NF] = hm8[b0:b0 + SPC].reshape(
            SPC, P, HMF).transpose(1, 0, 2).reshape(P, NF)
        pk[:, IN_XW + NF:IN_XW + 2 * NF] = w4_8[b0:b0 + SPC].reshape(
            SPC, P, HMF).transpose(1, 0, 2).reshape(P, NF)
        fr = np.zeros((P, FRW), np.float16)
        for s in range(SPC):
            fr[s * D:(s + 1) * D, s] = feat[b0 + s].astype(np.float16)
        fr[:, 2:4] = np.tile(rn[b0:b0 + SPC].astype(np.float16), (P, 1))
        in_maps.append({
            "inp8": pk,
            "fr16": fr,
            "cst_p": cp8[b0:b0 + SPC].reshape(P, HW),
        })
    return in_maps, host


def combine_outputs(results, host):
    """results: list of 8 per-core {'s_vec': [128,288] f16,
    'stats': [128,16] f32 per-partition partials}."""
    stc = np.stack([r["stats"].astype(np.float64).sum(axis=0)
                    for r in results])
    col = stc.sum(axis=0)                            # [16] over cores
    neg_s = col[ST_NEG]                  # device stores sum log(1-p)*p^2*w
    off_sq = col[ST_OFFSQ]
    off_n = col[ST_OFFN]
    cst_sq = col[ST_E2:ST_E2 + NG].sum()

    num_pos, pos_s = host["num_pos"], host["pos_s"]
    if num_pos == 0:
        loss_hm = -neg_s
    else:
        loss_hm = -(pos_s + neg_s) / max(num_pos, 1.0)
    svs = [r["s_vec"].reshape(-1).astype(np.float64) for r in results]
    h = B // 2
    s_pos = sum(svs[:NCORES // 2])
    s_neg = sum(svs[NCORES // 2:])
    loss_dst = 0.5 * (s_pos @ s_neg - s_pos @ s_pos) / (h * h) * 0.1
    loss_off = 0.5 * (off_sq / (B * 2 * HW)) / (off_n + 1e-6)
    loss_cst = cst_sq / (B * HW) * 0.1
    return np.array([loss_hm, host["loss_cls"], loss_dst, loss_off,
                     loss_cst, host["loss_tmp"]], np.float32)


_CACHE = {}


def kernel(**inputs):
    from concourse.bass_utils import run_bass_kernel_spmd
    if "nc" not in _CACHE:
        _CACHE["nc"] = build_nc()
    nc = _CACHE["nc"]
    in_maps, host = shard_inputs(**inputs)
    res = run_bass_kernel_spmd(nc, in_maps, core_ids=list(range(NCORES)))
    return combine_outputs(res.results, host)


# revision 43
# speedup vs baseline: 1.0374x; 1.0265x over previous
"""Trainium2 Bass kernel for the combined focal loss (8-core data parallel).

Sharding: batch dim B=16 split 2 samples/core across 8 cores. Each core
computes partial sums of the heavy loss terms; the host combines the
(tiny) partials in float64. The pairwise-cosine term is reduced
algebraically:

    pos_sum - neg_sum = 0.5 * (s_pos . s_neg - ||s_pos||^2)

with s_pos/s_neg sums of row-normalized flattened heatmaps, so each core
only returns its local unit-row sum [128, 288] (fp16) and no collective
is needed.

The dominant traffic is cstency_preds, downcast host-side to fp8-e4m3
(quantization rel-err ~2e-3 on the loss, an order under the 2e-2 gate).
It streams on the sync-engine HWDGE ring in 8 chunks sized so the tail
chunk is tiny (4 blocks): after the last byte lands, only ~4 matmuls +
one small sigmoid + two small vector ops + the stats DMA remain on the
critical path. Heatmaps ship as fp8 too: x raw plus (1-g)^4
precomputed on host (so the focal neg weight needs no device ops), with
feat/rn as a separate 4-col fp16 tensor. The scalar engine orders its
work Sigmoid/Sigmoid/Ln so only 3 ACT table loads happen and the
Sigmoid table is resident again before the stream sigmoids.

Partial sums ship raw as stats[128, 16] (host sums partitions) -- no
PE reduction on the tail. s_vec ships as two fp16 half-DMAs so no
single DMA queue carries the whole output (a straggler queue delayed
the stream by ~1 us in the 1-DMA f32 version).

The de-minimis scalar terms (cls/temporal BCE over 16x1/16x8 inputs,
and the focal pos-term, which touches only the ~2 elements/sample where
gt==1.0) are computed on host in float64, like the argmax/row-norm prep.
"""

import numpy as np
import ml_dtypes

import concourse.bacc as bacc
import concourse.tile as tile
from concourse import mybir

F32 = mybir.dt.float32
F16 = mybir.dt.float16
F8 = mybir.dt.float8e4
AF = mybir.ActivationFunctionType
ALU = mybir.AluOpType

B = 16
H = W = 192
HW = H * W            # 36864
D = 64
NCORES = 8
SPC = B // NCORES     # 2 samples per core
P = 128
HMF = HW // P         # 288 cols per sample in [128, .] layout
NF = SPC * HMF        # 576
OFFW = SPC * 2 * HW // P   # 1152 cols (2 samples x 2 ch)
FL_EPS = 0.1
NOISE = 0.2
POSC = (1.0 - FL_EPS) + FL_EPS * NOISE   # 0.92

# stream chunk sizes in 128-col blocks. Uniform small chunks: the
# per-chunk matmul batch trails its chunk's DMA by ~1 chunk, so big
# chunks pile up ~4 us of serialized matmuls after the stream ends;
# 18-block chunks keep the backlog under ~1 us while the sync-seq can
# still generate descriptors (0.62 us/chunk) faster than the queues
# drain them (0.83 us/chunk).
CHUNKS = [18] * 16
assert sum(CHUNKS) == HW // P
NG = len(CHUNKS)

# stats tile columns (per-partition partials; host sums partitions).
# The consistency term uses sum((enc-gt)^2) = sum(enc^2) - 2*sum(enc*gt)
# + sum(gt^2): enc^2 accumulates on the scalar engine (Square is in
# every ACT table, so no table reload), enc*gt on vector, gt^2 on host.
ST_NEG = 0      # sum log(1-p) * p^2 * (1-g)^4  == neg_s
ST_OFFSQ = 1    # sum ((p-g)*c)^2
ST_OFFN = 2     # sum c
ST_E2 = 3               # NG cols, per-chunk sum enc^2
ST_EG = ST_E2 + NG      # NG cols, per-chunk sum enc*gt
ST_W = 40
assert ST_EG + NG <= ST_W

FRW = 256       # feat/rn tensor width (padded, see fr16 below)


IN_GT = 0                      # cst gts, pretransposed [p, blk, s]
IN_OFF = IN_GT + HMF * SPC     # offset [preds | gts]
IN_XW = IN_OFF + 2 * OFFW      # heatmap [x | (1-g)^4]
IN_W = IN_XW + 2 * NF          # 4032 fp8 cols total


def build_nc():
    nc = bacc.Bacc(None, target_bir_lowering=False)

    # ALL fp8 inputs packed into one tensor [gt | off | xw] shipped as
    # the FIRST DMA on the sync ring, ahead of the stream chunks. The
    # 16 HW DMA queues round-robin between logical rings, so inputs on
    # any other ring get ~15% duty against the stream and land as late
    # as ~21 us (gating the tail). FIFO order on the stream's own ring
    # guarantees they land first, at full bandwidth (~1.3 us).
    inp8 = nc.dram_tensor("inp8", [P, IN_W], F8, kind="ExternalInput")
    # feat (2 cols, block-diag by sample) + rn (2 cols) + zero padding,
    # fp16 (padded to 512 B/partition to dodge the tiny-packet DMA
    # path); rides the otherwise-idle gpsimd SWDGE ring, needed only
    # by the matmuls/sv which have slack.
    fr16 = nc.dram_tensor("fr16", [P, FRW], F16, kind="ExternalInput")
    cst_p = nc.dram_tensor("cst_p", [P, HW], F8, kind="ExternalInput")

    s_vec = nc.dram_tensor("s_vec", [P, HMF], F16, kind="ExternalOutput")
    stats = nc.dram_tensor("stats", [P, ST_W], F32, kind="ExternalOutput")

    with tile.TileContext(nc, pool_alloc_mode="queue") as tc:
        with (
            tc.tile_pool(name="consts", bufs=1) as consts,
            tc.tile_pool(name="cstp", bufs=NG) as cstp,
            tc.tile_pool(name="encp", bufs=10) as encp,
            tc.tile_pool(name="hmp", bufs=1) as hmp,
            tc.tile_pool(name="offp", bufs=1) as offp,
            tc.tile_pool(name="ps_stream", bufs=8, space="PSUM") as ps_stream,
        ):
            # chunk0's DMA goes FIRST, even before the packed inputs:
            # the tensor engine is ~100% busy for 288 x ~48ns = 13.8 us
            # of back-to-back matmuls, so every us the first chunk lands
            # earlier moves the whole matmul stream (and the tail) up.
            # (An extra descriptor generation ahead of chunk0 delays the
            # whole stream by its ~0.6 us gen time, so nothing else may
            # precede it on the sync ring.)
            cw0 = CHUNKS[0] * P
            t_first = cstp.tile([P, cw0], F8, name="t0", tag="t")
            nc.sync.dma_start(out=t_first, in_=cst_p[:, 0:cw0])
            inp = hmp.tile([P, IN_W], F8)
            nc.sync.dma_start(out=inp, in_=inp8[:, :])
            fr = consts.tile([P, FRW], F16)
            nc.gpsimd.dma_start(out=fr, in_=fr16[:, :])

            st = consts.tile([P, ST_W], F32)

            gt_sb = inp[:, IN_GT:IN_GT + HMF * SPC]
            off_sb = inp[:, IN_OFF:IN_OFF + 2 * OFFW]
            xw = inp[:, IN_XW:IN_XW + 2 * NF]
            xf = xw[:, 0:NF]
            wf = xw[:, NF:2 * NF]
            feat_sb = fr[:, 0:2]
            rns_sb = fr[:, 2:4]
            w = {k: hmp.tile([P, NF], F16, tag=k, name=k)
                 for k in ("sp", "pt", "p2")}

            # scalar engine: both Sigmoids, then Ln, then the stream's
            # sigmoids -> 3 ACT table loads (no table holds both
            # Sigmoid and Ln), with the Sigmoid table resident again
            # before the stream so no mid-stream reload happens.
            # clip(p, 1e-4, 1-1e-4) is a no-op for |x| < 9.2, and
            # 1-p == sigmoid(-x) at table precision.
            nc.scalar.activation(w["sp"], xf, AF.Sigmoid, scale=-1.0)  # 1-p
            nc.scalar.activation(w["pt"], xf, AF.Sigmoid)              # p
            nc.scalar.activation(w["sp"], w["sp"], AF.Ln)       # log(1-p)

            # ---- offset partials, all on vector (everything finishes
            # by ~15 us, well before the stream tail needs the engine;
            # gpsimd is ~5x slower and would head-of-line-block oj) ----
            op_ = off_sb[:, :OFFW]
            og_ = off_sb[:, OFFW:]
            oc = offp.tile([P, OFFW], F16, tag="oc")
            nc.vector.tensor_scalar(
                out=oc, in0=og_, scalar1=0.0, scalar2=None, op0=ALU.is_gt,
            )                                                  # c
            nc.vector.reduce_sum(st[:, ST_OFFN:ST_OFFN + 1], oc[:],
                                 axis=mybir.AxisListType.X)
            od = offp.tile([P, OFFW], F16, tag="od")
            nc.vector.tensor_sub(od, op_, og_)                 # p - g
            om = offp.tile([P, OFFW], F16, tag="om")
            nc.vector.tensor_mul(om, od, oc)                   # (p-g)*c
            oj = offp.tile([P, OFFW], F16, tag="oj")
            nc.vector.scalar_tensor_tensor(
                out=oj, in0=om, scalar=1.0, in1=om,
                op0=ALU.mult, op1=ALU.mult,
                accum_out=st[:, ST_OFFSQ:ST_OFFSQ + 1],
            )

            # ---- heatmap focal neg partials, gated on Ln (~14 us); the
            # pos term and (1-g)^4 are host-side ----
            nc.vector.tensor_mul(w["p2"], w["pt"], w["pt"])    # p^2
            nc.vector.tensor_mul(w["p2"], w["sp"], w["p2"])    # sp*p^2
            nc.vector.scalar_tensor_tensor(
                out=w["p2"], in0=w["p2"], scalar=1.0, in1=wf,
                op0=ALU.mult, op1=ALU.mult,
                accum_out=st[:, ST_NEG:ST_NEG + 1],
            )

            # cosine partial: s_c = sum_s x_s * rn_s (ptr-scalar ops are
            # vector-only and need f32 scalars)
            rns32 = hmp.tile([P, SPC], F32, tag="rns32")
            nc.vector.tensor_copy(rns32, rns_sb)
            sv = hmp.tile([P, HMF], F16)
            nc.vector.tensor_scalar_mul(sv, xw[:, HMF:2 * HMF],
                                        rns32[:, 1:2])
            nc.vector.scalar_tensor_tensor(
                out=sv, in0=xw[:, 0:HMF], scalar=rns32[:, 0:1], in1=sv,
                op0=ALU.mult, op1=ALU.add,
            )

            # ---- consistency stream: sync ring, NG chunks ----
            col = 0
            for g, nb in enumerate(CHUNKS):
                cw = nb * P
                if g == 0:
                    t = t_first
                else:
                    t = cstp.tile([P, cw], F8, name="t%d" % g, tag="t")
                    nc.sync.dma_start(out=t, in_=cst_p[:, col:col + cw])
                pa = ps_stream.tile([P, nb, SPC], F32, tag="pa", name="pa")
                for j in range(nb):
                    nc.tensor.matmul(
                        pa[:, j, :], t[:, j * P:(j + 1) * P], feat_sb[:],
                        start=True, stop=True,
                    )
                enc = encp.tile([P, nb * SPC], F16, tag="enc", name="enc")
                nc.scalar.activation(
                    enc, pa.rearrange("p a b -> p (a b)"), AF.Sigmoid,
                    scale=0.125,
                )
                blk0 = col // P
                dif = encp.tile([P, nb * SPC], F16, tag="dif", name="dif")
                nc.vector.tensor_sub(
                    dif, enc, gt_sb[:, blk0 * SPC:(blk0 + nb) * SPC])
                dsq = encp.tile([P, nb * SPC], F16, tag="dsq", name="dsq")
                nc.vector.scalar_tensor_tensor(
                    out=dsq, in0=dif, scalar=1.0, in1=dif,
                    op0=ALU.mult, op1=ALU.mult,
                    accum_out=st[:, ST_E2 + g:ST_E2 + g + 1],
                )
                col += cw

            # s_vec ships AFTER the stream-sigmoid emissions: a
            # dma_start stalls its whole sequencer until the data is
            # ready, so putting these last on the scalar ring means
            # they can't delay the ACT table loads / stream sigmoids.
            # Two half-DMAs so no single queue carries the whole output.
            hh = HMF // 2
            nc.scalar.dma_start(out=s_vec[:, 0:hh], in_=sv[:, 0:hh])
            nc.scalar.dma_start(out=s_vec[:, hh:HMF], in_=sv[:, hh:HMF])

            # ship the raw per-partition stats; host sums the 128 rows
            nc.sync.dma_start(out=stats[:, :], in_=st)

    nc.finalize()
    return nc


def shard_inputs(hm_outputs, hm_targets, cls_preds, cls_gts,
                 offset_preds, offset_gts, cstency_preds, cstency_gts,
                 temp_loc_preds, temp_loc_gts):
    """Build the 8 per-core input maps + host-side fp64 scalar terms."""
    hm = np.ascontiguousarray(hm_outputs, np.float32).reshape(B, HW)
    hg = np.ascontiguousarray(hm_targets, np.float32).reshape(B, HW)
    hm8 = hm.astype(ml_dtypes.float8_e4m3)
    w4_8 = ((1.0 - hg) ** 4).astype(ml_dtypes.float8_e4m3)
    off = np.concatenate([
        np.ascontiguousarray(offset_preds, np.float32).reshape(B, 2 * HW),
        np.ascontiguousarray(offset_gts, np.float32).reshape(B, 2 * HW),
    ], axis=1).astype(ml_dtypes.float8_e4m3)     # [B, 4*HW] = [p | g]
    cp = np.ascontiguousarray(cstency_preds, np.float32).reshape(B, D, HW)
    cg = np.ascontiguousarray(cstency_gts, np.float32).reshape(B, HW)

    # rn from the QUANTIZED x so device unit-rows are self-consistent
    hm8f = hm8.astype(np.float64)
    rn = (1.0 / np.maximum(np.sqrt((hm8f ** 2).sum(axis=1)),
                           1e-6)).astype(np.float32)
    idx = np.argmax(cg, axis=-1)                       # [B]
    feat = cp[np.arange(B), :, idx]                    # [B, D] peak features
    cp8 = cp.astype(ml_dtypes.float8_e4m3)

    # ---- host fp64 de-minimis terms ----
    def bce_mean(x, y):
        x = x.astype(np.float64).ravel()
        y = y.astype(np.float64).ravel()
        sp = np.log1p(np.exp(-np.abs(x))) + np.maximum(x, 0.0)
        return float((sp - x * y).mean())

    loss_cls = bce_mean(np.asarray(cls_preds), np.asarray(cls_gts))
    loss_tmp = bce_mean(np.asarray(temp_loc_preds), np.asarray(temp_loc_gts))

    pos_mask = hg == 1.0
    num_pos = float(pos_mask.sum())
    xp = hm[pos_mask].astype(np.float64)
    pp = np.clip(1.0 / (1.0 + np.exp(-xp)), 1e-4, 1.0 - 1e-4)
    pos_s = float((POSC * np.log(pp) * (1.0 - pp) ** 2).sum())

    # sum(gt^2) over the SAME fp8-quantized gts the device sees, so the
    # expansion sum(enc^2) - 2*sum(enc*gt) + sum(gt^2) is consistent
    gt2 = float((cg.astype(ml_dtypes.float8_e4m3)
                 .astype(np.float64) ** 2).sum())

    host = {"loss_cls": loss_cls, "loss_tmp": loss_tmp,
            "num_pos": num_pos, "pos_s": pos_s, "gt2": gt2}

    in_maps = []
    for c in range(NCORES):
        b0 = c * SPC
        # packed fp8 inputs [gt | off | xw]; gt pre-transposed to the
        # matmul output layout: gt[p, blk*SPC + s] = cg[b0+s, blk*128+p]
        pk = np.empty((P, IN_W), ml_dtypes.float8_e4m3)
        pk[:, IN_GT:IN_GT + HMF * SPC] = np.ascontiguousarray(
            cg[b0:b0 + SPC].reshape(SPC, HMF, P).transpose(2, 1, 0)
        ).reshape(P, HMF * SPC).astype(ml_dtypes.float8_e4m3)
        pk[:, IN_OFF:IN_OFF + 2 * OFFW] = off[b0:b0 + SPC].reshape(
            P, 2 * OFFW)
        pk[:, IN_XW:IN_XW + NF] = hm8[b0:b0 + SPC].reshape(
            SPC, P, HMF).transpose(1, 0, 2).reshape(P, NF)
        pk[:, IN_XW + NF:IN_XW + 2 * NF] = w4_8[b0:b0 + SPC].reshape(
            SPC, P, HMF).transpose(1, 0, 2).reshape(P, NF)
        fr = np.zeros((P, FRW), np.float16)
        for s in range(SPC):
            fr[s * D:(s + 1) * D, s] = feat[b0 + s].astype(np.float16)
        fr[:, 2:4] = np.tile(rn[b0:b0 + SPC].astype(np.float16), (P, 1))
        in_maps.append({
            "inp8": pk,
            "fr16": fr,
            "cst_p": cp8[b0:b0 + SPC].reshape(P, HW),
        })
    return in_maps, host


def combine_outputs(results, host):
    """results: list of 8 per-core {'s_vec': [128,288] f16,
    'stats': [128,16] f32 per-partition partials}."""
    stc = np.stack([r["stats"].astype(np.float64).sum(axis=0)
                    for r in results])
    col = stc.sum(axis=0)                            # [16] over cores
    neg_s = col[ST_NEG]                  # device stores sum log(1-p)*p^2*w
    off_sq = col[ST_OFFSQ]
    off_n = col[ST_OFFN]
    cst_sq = col[ST_E2:ST_E2 + NG].sum()

    num_pos, pos_s = host["num_pos"], host["pos_s"]
    if num_pos == 0:
        loss_hm = -neg_s
    else:
        loss_hm = -(pos_s + neg_s) / max(num_pos, 1.0)
    svs = [r["s_vec"].reshape(-1).astype(np.float64) for r in results]
    h = B // 2
    s_pos = sum(svs[:NCORES // 2])
    s_neg = sum(svs[NCORES // 2:])
    loss_dst = 0.5 * (s_pos @ s_neg - s_pos @ s_pos) / (h * h) * 0.1
    loss_off = 0.5 * (off_sq / (B * 2 * HW)) / (off_n + 1e-6)
    loss_cst = cst_sq / (B * HW) * 0.1
    return np.array([loss_hm, host["loss_cls"], loss_dst, loss_off,
                     loss_cst, host["loss_tmp"]], np.float32)


_CACHE = {}


def kernel(**inputs):
    from concourse.bass_utils import run_bass_kernel_spmd
    if "nc" not in _CACHE:
        _CACHE["nc"] = build_nc()
    nc = _CACHE["nc"]
    in_maps, host = shard_inputs(**inputs)
    res = run_bass_kernel_spmd(nc, in_maps, core_ids=list(range(NCORES)))
    return combine_outputs(res.results, host)
